# revision 24
# baseline (speedup 1.0000x reference)
"""Trainium2 Bass kernel for LoRA-segmented linear layer.

Computes y = x @ W^T + bias + scalings[e] * (x_e @ A_e^T) @ B_e^T
where x is split into 8 equal contiguous token segments (one per adapter).

Sharding: data-parallel over tokens; core e gets segment e (4096 tokens),
which exactly matches adapter e, so all LoRA work is core-local.

The LoRA fold W_eff^T = W^T + A_e^T @ (s_e * B_e^T) is precomputed on the
host in fp32 (rank-16 update, ~0.1% of the total FLOPs) so the device
kernel is a single dense GEMM:

  y_e = x_e @ W_eff^T + bias

Precision split (tolerance 2e-2; measured rel err ~1.35e-2):
  - k-rows 0:256 run as one fp8e4 DoubleRow matmul per (m, oc) group
    (2 k-tiles in one PE pass = ~2x on that slice)
  - k-rows 256:2048 run in bf16 at the 78.6 TF/s roofline
  - weff/bias are pre-scaled by 64 on the host so weff's ~0.02-magnitude
    values sit in e4m3's normal range; the host divides y by 64 (exact)

Schedule (from trace analysis):
  - warmup matmuls on a zeroed tile lift the PE clock gate (HAM 4/8 ->
    8/8) while the first real DMAs are still in flight
  - chunk-0 consumes weff k-tiles in DMA arrival order (k-outer over an
    m-pair = 8 PSUM banks) so PE demand matches DMA supply
  - later groups run oc-outer so each oc's bias-add + output DMA overlap
    the next oc's matmuls, and the kernel tail is one add + one 256 KB DMA
"""

import numpy as np
import ml_dtypes

# Problem geometry (hardcoded per contest contract).
N_TOK, D_IN, D_OUT, E, R = 32768, 2048, 2048, 8, 16
S = N_TOK // E          # tokens per core / segment: 4096
P = 128                 # partitions
NPAIR = 2               # fp8 DoubleRow k-pairs (256 rows each)
KF = NPAIR * 256        # fp8 contraction rows
NKB = (D_IN - KF) // P  # bf16 contraction tiles
TCH = 512               # token chunk (x dma width)
NCH = S // TCH          # 8 token chunks per core
M_PER = TCH // P        # 4 m-subtiles (of 128 tokens) per chunk
OC = 512                # dout chunk (matmul moving free dim; one PSUM bank)
NOC = D_OUT // OC       # 4 dout chunks
N_WARM = 10             # dummy matmuls to lift the PE clock gate (~4us)
WSCALE = 64.0           # pow-2 pre-scale keeping weff out of e4m3 subnormals

_PROGRAM = None         # cached Bass program
LAST_RESULTS = None     # BassKernelResults of the most recent run (for profiling)


def _build_program():
    from contextlib import ExitStack

    import concourse.mybir as mybir
    import concourse.tile as tile
    from concourse import bacc

    bf = mybir.dt.bfloat16
    f8 = mybir.dt.float8e4
    f32 = mybir.dt.float32
    DR = mybir.MatmulPerfMode.DoubleRow

    nc = bacc.Bacc(trn_type="TRN2")

    # bf16 operands cover k-rows KF..D_IN; fp8 pair covers rows 0..KF.
    xt = nc.dram_tensor("xt", [D_IN - KF, S], bf, kind="ExternalInput")
    weff_d = nc.dram_tensor("weff", [D_IN - KF, D_OUT], bf, kind="ExternalInput")
    x8_d = nc.dram_tensor("x8", [P, 2 * NPAIR, S], f8, kind="ExternalInput")
    w8_d = nc.dram_tensor("w8", [P, 2 * NPAIR, D_OUT], f8, kind="ExternalInput")
    bias_d = nc.dram_tensor("bias", [D_OUT], f32, kind="ExternalInput")
    y = nc.dram_tensor("y", [S, D_OUT], f32, kind="ExternalOutput")

    with ExitStack() as ctx:
        tc = ctx.enter_context(tile.TileContext(nc))
        persist = ctx.enter_context(tc.tile_pool(name="persist", bufs=1))
        xp = ctx.enter_context(tc.tile_pool(name="xp", bufs=28))
        x8p = ctx.enter_context(tc.tile_pool(name="x8p", bufs=3))
        outp = ctx.enter_context(tc.tile_pool(name="outp", bufs=8))
        psum = ctx.enter_context(tc.tile_pool(name="psum", bufs=8, space="PSUM"))

        # --- PE warmup: zero tile + dummy matmuls so the HAM clock gate
        # reaches 8/8 while the first real DMAs are still in flight ---
        warm = persist.tile([P, OC], bf, tag="warm", name="warm")
        nc.vector.memset(warm, 0.0)
        wps = psum.tile([P, OC], f32, tag="ps", name="warm_ps")
        for i in range(N_WARM):
            nc.tensor.matmul(wps, warm[:, :P], warm, start=True, stop=True)

        # --- head DMAs, in consumption order. The first two transfers are
        # exactly the operands of the first DR matmul (m=0 token sliver of
        # x8 + w8 oc0), so the PE can start ~3us sooner than with whole-tile
        # transfers; the rest stream in behind them. ---
        x8c0 = x8p.tile([P, 2 * NPAIR, TCH], f8, tag="x8", name="x8_0")
        nc.sync.dma_start(out=x8c0[:, :, 0:P], in_=x8_d[:, :, 0:P])
        w8_sb = persist.tile([P, 2 * NPAIR, D_OUT], f8, tag="w8", name="w8_sb")
        nc.sync.dma_start(out=w8_sb[:, :, 0:OC], in_=w8_d[:, :, 0:OC])
        nc.sync.dma_start(out=w8_sb[:, :, OC:2 * OC], in_=w8_d[:, :, OC:2 * OC])
        nc.sync.dma_start(out=x8c0[:, :, P:TCH], in_=x8_d[:, :, P:TCH])
        nc.sync.dma_start(out=w8_sb[:, :, 2 * OC:3 * OC], in_=w8_d[:, :, 2 * OC:3 * OC])
        nc.sync.dma_start(out=w8_sb[:, :, 3 * OC:4 * OC], in_=w8_d[:, :, 3 * OC:4 * OC])
        x0 = []
        weff = []
        for k in range(NKB):
            xkt = xp.tile([P, TCH], bf, tag="xk", name=f"xk_0_{k}")
            nc.sync.dma_start(out=xkt, in_=xt[k * P:(k + 1) * P, 0:TCH])
            x0.append(xkt)
            we = persist.tile([P, D_OUT], bf, tag=f"weff{k}", name=f"weff_{k}")
            nc.sync.dma_start(out=we, in_=weff_d[k * P:(k + 1) * P, :])
            weff.append(we)
        bias_sb = persist.tile([P, D_OUT], f32, tag="bias", name="bias_sb")
        # stride-0 partition broadcast must go via SW DGE (gpsimd), not HW DGE
        nc.gpsimd.dma_start(out=bias_sb, in_=bias_d[:].partition_broadcast(P))

        def mm_pair(ps, x8c, m, oc, pair, start):
            nc.tensor.matmul(
                ps,
                x8c[:, 2 * pair:2 * pair + 2, m * P:(m + 1) * P],
                w8_sb[:, 2 * pair:2 * pair + 2, oc * OC:(oc + 1) * OC],
                start=start,
                stop=False,
                perf_mode=DR,
            )

        def mm_bf(ps, xk, m, k, oc, stop=False):
            nc.tensor.matmul(
                ps,
                xk[k][:, m * P:(m + 1) * P],
                weff[k][:, oc * OC:(oc + 1) * OC],
                start=False,
                stop=stop,
            )

        def emit_oc_out(t, m, oc, ps):
            row0 = (t * M_PER + m) * P
            ob = outp.tile([P, OC], f32, tag="ob", name=f"ob_{t}_{m}_{oc}")
            nc.vector.tensor_add(ob, ps, bias_sb[:, oc * OC:(oc + 1) * OC])
            # outputs issue on the Scalar HWDGE ring, decoupled from input
            # prefetch issue on Sync
            nc.scalar.dma_start(
                out=y[row0:row0 + P, oc * OC:(oc + 1) * OC], in_=ob
            )

        # --- chunk 0, m-pair (0,1): k-outer so weff is consumed in DMA
        # arrival order (8 matmuls per k-tile ~ DMA supply rate) ---
        pss = {
            m: [
                psum.tile([P, OC], f32, tag="ps", name=f"ps_0_{m}_{oc}")
                for oc in range(NOC)
            ]
            for m in (0, 1)
        }
        for m in (0, 1):
            for pair in range(NPAIR):
                for oc in range(NOC):
                    mm_pair(pss[m][oc], x8c0, m, oc, pair, start=(pair == 0))
        for k in range(NKB):
            for m in (0, 1):
                lhsT = x0[k][:, m * P:(m + 1) * P]
                for oc in range(NOC):
                    nc.tensor.matmul(
                        pss[m][oc],
                        lhsT,
                        weff[k][:, oc * OC:(oc + 1) * OC],
                        start=False,
                        stop=(k == NKB - 1),
                    )
        for m in (0, 1):
            for oc in range(NOC):
                emit_oc_out(0, m, oc, pss[m][oc])

        def emit_group(t, m, x8c, xk):
            # k-outer / oc-inner: 4 consecutive matmuls share one stationary
            # operand so LDWEIGHTS amortizes (oc-outer measured +42ns/MM)
            pss_m = [
                psum.tile([P, OC], f32, tag="ps", name=f"ps_{t}_{m}_{oc}")
                for oc in range(NOC)
            ]
            for pair in range(NPAIR):
                for oc in range(NOC):
                    mm_pair(pss_m[oc], x8c, m, oc, pair, start=(pair == 0))
            for k in range(NKB):
                for oc in range(NOC):
                    mm_bf(pss_m[oc], xk, m, k, oc, stop=(k == NKB - 1))
            for oc in range(NOC):
                emit_oc_out(t, m, oc, pss_m[oc])

        for m in (2, 3):
            emit_group(0, m, x8c0, x0)

        # --- remaining token chunks ---
        for t in range(1, NCH):
            x8c = x8p.tile([P, 2 * NPAIR, TCH], f8, tag="x8", name=f"x8_{t}")
            nc.sync.dma_start(out=x8c, in_=x8_d[:, :, t * TCH:(t + 1) * TCH])
            xk = []
            for k in range(NKB):
                xkt = xp.tile([P, TCH], bf, tag="xk", name=f"xk_{t}_{k}")
                nc.sync.dma_start(
                    out=xkt, in_=xt[k * P:(k + 1) * P, t * TCH:(t + 1) * TCH]
                )
                xk.append(xkt)
            for m in range(M_PER):
                emit_group(t, m, x8c, xk)

    return nc


def _get_program():
    global _PROGRAM
    if _PROGRAM is None:
        _PROGRAM = _build_program()
        # run_bass_via_pjrt does not finalize; Bacc's compile passes
        # (register alloc, wait legalization) run here.
        _PROGRAM.finalize()
    return _PROGRAM


def kernel(x, W, bias, lora_a, lora_b, scalings, trace=False):
    global LAST_RESULTS
    from concourse.bass_utils import run_bass_kernel_spmd

    assert x.shape == (N_TOK, D_IN) and W.shape == (D_OUT, D_IN)
    bf16 = ml_dtypes.bfloat16
    e4m3 = ml_dtypes.float8_e4m3
    f32 = np.float32

    # Host-side layout prep (not on the device critical path).
    xT = np.ascontiguousarray(x.T)                                 # [D_IN, N] f32
    # Fold the rank-16 LoRA update into the weight in fp32, round once:
    # weffT_e = W^T + A_e^T @ (s_e * B_e^T), pre-scaled by WSCALE (pow 2).
    a_t = np.ascontiguousarray(lora_a.transpose(0, 2, 1)).astype(f32)   # [E, D_IN, R]
    sb_t = np.ascontiguousarray(
        (lora_b.astype(np.float64) * scalings[:, None, None].astype(np.float64))
        .transpose(0, 2, 1)
    ).astype(f32)                                                  # [E, R, D_OUT]
    weffT = (W.T.astype(f32)[None, :, :] + np.matmul(a_t, sb_t)) * WSCALE
    weffT_bf = weffT[:, KF:, :].astype(bf16)                       # [E, D_IN-KF, D_OUT]
    # fp8 pair operands, laid out [128, 2*NPAIR, cols] for DoubleRow
    # (k = pair*256 + plane*128 + p)
    w8 = np.ascontiguousarray(
        weffT[:, :KF, :].reshape(E, 2 * NPAIR, P, D_OUT).transpose(0, 2, 1, 3)
    ).astype(e4m3)                                                 # [E, P, 2*NPAIR, D_OUT]
    x8 = np.ascontiguousarray(
        xT[:KF, :].reshape(2 * NPAIR, P, N_TOK).transpose(1, 0, 2)
    ).astype(e4m3)                                                 # [P, 2*NPAIR, N]
    xT_bf = xT[KF:, :].astype(bf16)                                # [D_IN-KF, N]
    bias32 = np.ascontiguousarray(bias.astype(f32) * WSCALE)

    in_maps = []
    for e in range(E):
        in_maps.append(
            {
                "xt": np.ascontiguousarray(xT_bf[:, e * S:(e + 1) * S]),
                "weff": np.ascontiguousarray(weffT_bf[e]),
                "x8": np.ascontiguousarray(x8[:, :, e * S:(e + 1) * S]),
                "w8": np.ascontiguousarray(w8[e]),
                "bias": bias32,
            }
        )

    nc = _get_program()
    res = run_bass_kernel_spmd(nc, in_maps, core_ids=list(range(E)), trace=trace)
    LAST_RESULTS = res
    out = np.concatenate([r["y"] for r in res.results], axis=0)
    return (out / np.float32(WSCALE)).astype(np.float32)


# revision 26
# speedup vs baseline: 1.0088x; 1.0088x over previous
"""Trainium2 Bass kernel for LoRA-segmented linear layer.

Computes y = x @ W^T + bias + scalings[e] * (x_e @ A_e^T) @ B_e^T
where x is split into 8 equal contiguous token segments (one per adapter).

Sharding: data-parallel over tokens; core e gets segment e (4096 tokens),
which exactly matches adapter e, so all LoRA work is core-local.

The LoRA fold W_eff^T = W^T + A_e^T @ (s_e * B_e^T) is precomputed on the
host in fp32 (rank-16 update, ~0.1% of the total FLOPs) so the device
kernel is a single dense GEMM:

  y_e = x_e @ W_eff^T + bias

Precision split (tolerance 2e-2; measured rel err ~1.35e-2):
  - k-rows 0:256 run as one fp8e4 DoubleRow matmul per (m, oc) group
    (2 k-tiles in one PE pass = ~2x on that slice)
  - k-rows 256:2048 run in bf16 at the 78.6 TF/s roofline
  - weff/bias are pre-scaled by 64 on the host so weff's ~0.02-magnitude
    values sit in e4m3's normal range; the host divides y by 64 (exact)

Schedule (from trace analysis):
  - warmup matmuls on a zeroed tile lift the PE clock gate (HAM 4/8 ->
    8/8) while the first real DMAs are still in flight
  - chunk-0 consumes weff k-tiles in DMA arrival order (k-outer over an
    m-pair = 8 PSUM banks) so PE demand matches DMA supply
  - later groups run oc-outer so each oc's bias-add + output DMA overlap
    the next oc's matmuls, and the kernel tail is one add + one 256 KB DMA
"""

import numpy as np
import ml_dtypes

# Problem geometry (hardcoded per contest contract).
N_TOK, D_IN, D_OUT, E, R = 32768, 2048, 2048, 8, 16
S = N_TOK // E          # tokens per core / segment: 4096
P = 128                 # partitions
NPAIR = 2               # fp8 DoubleRow k-pairs (256 rows each)
KF = NPAIR * 256        # fp8 contraction rows
NKB = (D_IN - KF) // P  # bf16 contraction tiles
TCH = 512               # token chunk (x dma width)
NCH = S // TCH          # 8 token chunks per core
M_PER = TCH // P        # 4 m-subtiles (of 128 tokens) per chunk
OC = 512                # dout chunk (matmul moving free dim; one PSUM bank)
NOC = D_OUT // OC       # 4 dout chunks
N_WARM = 16             # dummy matmuls to lift the PE clock gate; sized to
                        # bridge exactly to first-DMA-ready (~14.5us, a floor
                        # set by issue chain + HBM completion receipt)
WSCALE = 64.0           # pow-2 pre-scale keeping weff out of e4m3 subnormals

_PROGRAM = None         # cached Bass program
LAST_RESULTS = None     # BassKernelResults of the most recent run (for profiling)


def _build_program():
    from contextlib import ExitStack

    import concourse.mybir as mybir
    import concourse.tile as tile
    from concourse import bacc

    bf = mybir.dt.bfloat16
    f8 = mybir.dt.float8e4
    f32 = mybir.dt.float32
    DR = mybir.MatmulPerfMode.DoubleRow

    nc = bacc.Bacc(trn_type="TRN2")

    # bf16 operands cover k-rows KF..D_IN; fp8 pair covers rows 0..KF.
    xt = nc.dram_tensor("xt", [D_IN - KF, S], bf, kind="ExternalInput")
    weff_d = nc.dram_tensor("weff", [D_IN - KF, D_OUT], bf, kind="ExternalInput")
    x8_d = nc.dram_tensor("x8", [P, 2 * NPAIR, S], f8, kind="ExternalInput")
    w8_d = nc.dram_tensor("w8", [P, 2 * NPAIR, D_OUT], f8, kind="ExternalInput")
    bias_d = nc.dram_tensor("bias", [D_OUT], f32, kind="ExternalInput")
    y = nc.dram_tensor("y", [S, D_OUT], f32, kind="ExternalOutput")

    with ExitStack() as ctx:
        tc = ctx.enter_context(tile.TileContext(nc))
        persist = ctx.enter_context(tc.tile_pool(name="persist", bufs=1))
        xp = ctx.enter_context(tc.tile_pool(name="xp", bufs=28))
        x8p = ctx.enter_context(tc.tile_pool(name="x8p", bufs=3))
        outp = ctx.enter_context(tc.tile_pool(name="outp", bufs=8))
        psum = ctx.enter_context(tc.tile_pool(name="psum", bufs=8, space="PSUM"))

        # --- PE warmup: zero tile + dummy matmuls so the HAM clock gate
        # reaches 8/8 while the first real DMAs are still in flight ---
        warm = persist.tile([P, OC], bf, tag="warm", name="warm")
        nc.vector.memset(warm, 0.0)
        wps = psum.tile([P, OC], f32, tag="ps", name="warm_ps")
        for i in range(N_WARM):
            nc.tensor.matmul(wps, warm[:, :P], warm, start=True, stop=True)

        # --- head DMAs, in consumption order (whole tiles: splitting them
        # into slivers does not beat the ~14.5us issue+receipt floor and
        # only delays the weff stream behind extra issue slots) ---
        x8c0 = x8p.tile([P, 2 * NPAIR, TCH], f8, tag="x8", name="x8_0")
        nc.sync.dma_start(out=x8c0, in_=x8_d[:, :, 0:TCH])
        w8_sb = persist.tile([P, 2 * NPAIR, D_OUT], f8, tag="w8", name="w8_sb")
        for oc in range(NOC):
            nc.sync.dma_start(
                out=w8_sb[:, :, oc * OC:(oc + 1) * OC],
                in_=w8_d[:, :, oc * OC:(oc + 1) * OC],
            )
        x0 = []
        weff = []
        for k in range(NKB):
            xkt = xp.tile([P, TCH], bf, tag="xk", name=f"xk_0_{k}")
            nc.sync.dma_start(out=xkt, in_=xt[k * P:(k + 1) * P, 0:TCH])
            x0.append(xkt)
            we = persist.tile([P, D_OUT], bf, tag=f"weff{k}", name=f"weff_{k}")
            nc.sync.dma_start(out=we, in_=weff_d[k * P:(k + 1) * P, :])
            weff.append(we)
        bias_sb = persist.tile([P, D_OUT], f32, tag="bias", name="bias_sb")
        # stride-0 partition broadcast must go via SW DGE (gpsimd), not HW DGE
        nc.gpsimd.dma_start(out=bias_sb, in_=bias_d[:].partition_broadcast(P))

        def mm_pair(ps, x8c, m, oc, pair, start):
            nc.tensor.matmul(
                ps,
                x8c[:, 2 * pair:2 * pair + 2, m * P:(m + 1) * P],
                w8_sb[:, 2 * pair:2 * pair + 2, oc * OC:(oc + 1) * OC],
                start=start,
                stop=False,
                perf_mode=DR,
            )

        def mm_bf(ps, xk, m, k, oc, stop=False):
            nc.tensor.matmul(
                ps,
                xk[k][:, m * P:(m + 1) * P],
                weff[k][:, oc * OC:(oc + 1) * OC],
                start=False,
                stop=stop,
            )

        def emit_oc_out(t, m, oc, ps):
            row0 = (t * M_PER + m) * P
            ob = outp.tile([P, OC], f32, tag="ob", name=f"ob_{t}_{m}_{oc}")
            nc.vector.tensor_add(ob, ps, bias_sb[:, oc * OC:(oc + 1) * OC])
            # outputs issue on the Scalar HWDGE ring, decoupled from input
            # prefetch issue on Sync
            nc.scalar.dma_start(
                out=y[row0:row0 + P, oc * OC:(oc + 1) * OC], in_=ob
            )

        # --- chunk 0, m-pair (0,1): k-outer so weff is consumed in DMA
        # arrival order (8 matmuls per k-tile ~ DMA supply rate) ---
        pss = {
            m: [
                psum.tile([P, OC], f32, tag="ps", name=f"ps_0_{m}_{oc}")
                for oc in range(NOC)
            ]
            for m in (0, 1)
        }
        for m in (0, 1):
            for pair in range(NPAIR):
                for oc in range(NOC):
                    mm_pair(pss[m][oc], x8c0, m, oc, pair, start=(pair == 0))
        for k in range(NKB):
            for m in (0, 1):
                lhsT = x0[k][:, m * P:(m + 1) * P]
                for oc in range(NOC):
                    nc.tensor.matmul(
                        pss[m][oc],
                        lhsT,
                        weff[k][:, oc * OC:(oc + 1) * OC],
                        start=False,
                        stop=(k == NKB - 1),
                    )
        for m in (0, 1):
            for oc in range(NOC):
                emit_oc_out(0, m, oc, pss[m][oc])

        def emit_group(t, m, x8c, xk):
            # k-outer / oc-inner: 4 consecutive matmuls share one stationary
            # operand so LDWEIGHTS amortizes (oc-outer measured +42ns/MM)
            pss_m = [
                psum.tile([P, OC], f32, tag="ps", name=f"ps_{t}_{m}_{oc}")
                for oc in range(NOC)
            ]
            for pair in range(NPAIR):
                for oc in range(NOC):
                    mm_pair(pss_m[oc], x8c, m, oc, pair, start=(pair == 0))
            for k in range(NKB):
                for oc in range(NOC):
                    mm_bf(pss_m[oc], xk, m, k, oc, stop=(k == NKB - 1))
            for oc in range(NOC):
                emit_oc_out(t, m, oc, pss_m[oc])

        for m in (2, 3):
            emit_group(0, m, x8c0, x0)

        # --- remaining token chunks ---
        for t in range(1, NCH):
            x8c = x8p.tile([P, 2 * NPAIR, TCH], f8, tag="x8", name=f"x8_{t}")
            nc.sync.dma_start(out=x8c, in_=x8_d[:, :, t * TCH:(t + 1) * TCH])
            xk = []
            for k in range(NKB):
                xkt = xp.tile([P, TCH], bf, tag="xk", name=f"xk_{t}_{k}")
                nc.sync.dma_start(
                    out=xkt, in_=xt[k * P:(k + 1) * P, t * TCH:(t + 1) * TCH]
                )
                xk.append(xkt)
            for m in range(M_PER):
                emit_group(t, m, x8c, xk)

    return nc


def _get_program():
    global _PROGRAM
    if _PROGRAM is None:
        _PROGRAM = _build_program()
        # run_bass_via_pjrt does not finalize; Bacc's compile passes
        # (register alloc, wait legalization) run here.
        _PROGRAM.finalize()
    return _PROGRAM


def kernel(x, W, bias, lora_a, lora_b, scalings, trace=False):
    global LAST_RESULTS
    from concourse.bass_utils import run_bass_kernel_spmd

    assert x.shape == (N_TOK, D_IN) and W.shape == (D_OUT, D_IN)
    bf16 = ml_dtypes.bfloat16
    e4m3 = ml_dtypes.float8_e4m3
    f32 = np.float32

    # Host-side layout prep (not on the device critical path).
    xT = np.ascontiguousarray(x.T)                                 # [D_IN, N] f32
    # Fold the rank-16 LoRA update into the weight in fp32, round once:
    # weffT_e = W^T + A_e^T @ (s_e * B_e^T), pre-scaled by WSCALE (pow 2).
    a_t = np.ascontiguousarray(lora_a.transpose(0, 2, 1)).astype(f32)   # [E, D_IN, R]
    sb_t = np.ascontiguousarray(
        (lora_b.astype(np.float64) * scalings[:, None, None].astype(np.float64))
        .transpose(0, 2, 1)
    ).astype(f32)                                                  # [E, R, D_OUT]
    weffT = (W.T.astype(f32)[None, :, :] + np.matmul(a_t, sb_t)) * WSCALE
    weffT_bf = weffT[:, KF:, :].astype(bf16)                       # [E, D_IN-KF, D_OUT]
    # fp8 pair operands, laid out [128, 2*NPAIR, cols] for DoubleRow
    # (k = pair*256 + plane*128 + p)
    w8 = np.ascontiguousarray(
        weffT[:, :KF, :].reshape(E, 2 * NPAIR, P, D_OUT).transpose(0, 2, 1, 3)
    ).astype(e4m3)                                                 # [E, P, 2*NPAIR, D_OUT]
    x8 = np.ascontiguousarray(
        xT[:KF, :].reshape(2 * NPAIR, P, N_TOK).transpose(1, 0, 2)
    ).astype(e4m3)                                                 # [P, 2*NPAIR, N]
    xT_bf = xT[KF:, :].astype(bf16)                                # [D_IN-KF, N]
    bias32 = np.ascontiguousarray(bias.astype(f32) * WSCALE)

    in_maps = []
    for e in range(E):
        in_maps.append(
            {
                "xt": np.ascontiguousarray(xT_bf[:, e * S:(e + 1) * S]),
                "weff": np.ascontiguousarray(weffT_bf[e]),
                "x8": np.ascontiguousarray(x8[:, :, e * S:(e + 1) * S]),
                "w8": np.ascontiguousarray(w8[e]),
                "bias": bias32,
            }
        )

    nc = _get_program()
    res = run_bass_kernel_spmd(nc, in_maps, core_ids=list(range(E)), trace=trace)
    LAST_RESULTS = res
    out = np.concatenate([r["y"] for r in res.results], axis=0)
    return (out / np.float32(WSCALE)).astype(np.float32)
